# revision 15
# baseline (speedup 1.0000x reference)
"""Bahdanau attention TRN2 Bass kernel.

Shapes (hardcoded): B=32, S=2048, ENC=DEC=1024, fp32.
Sharding: data-parallel over batch B across 8 NeuronCores (4 batches/core);
W_h / W_s / v replicated.

Per-core algorithm (b in 0..3):
  enc_T[d, s]  = sum_e W_h[d, e] * encoder[b, s, e]        (PE, fp32r)
                 encoder tiles transposed on-chip via PE transpose-mode
  t[d, s]      = tanh(enc_T[d, s] + dec[b, d])             (ACT, fused bias)
  energy[s]    = sum_d v[d] * t[d, s]                      (PE, M=1 matmul)
  w[s]         = exp(energy - max) * mask                  (ACT + DVE fused reduce)
  attn[s]      = w[s] / sum(w)                             (DVE)
  context[e]   = (sum_s w[s] * encoder[b, s, e]) / sum(w)  (PE, M=1 matmul)
"""

import numpy as np

try:
    import concourse.bacc as bacc
except ImportError:  # fallback when the axon site path isn't preloaded
    import sys
    sys.path.insert(0, "/opt/trn_rl_repo")
    import concourse.bacc as bacc
import concourse.bass as bass
import concourse.mybir as mybir
import concourse.tile as tile
from concourse.bass_utils import run_bass_kernel_spmd
from concourse.masks import make_identity

B, S, E, D = 32, 2048, 1024, 1024
NCORES = 8
BLOC = B // NCORES  # 4 batches per core
P = 128
SCH = 512           # s-chunk processed per inner iteration
NCHUNK = S // SCH   # 4
ET = E // P         # 8 e-tiles (contraction tiles for the big matmul)
DT = D // P         # 8 d-tiles
ST = S // P         # 16 s-tiles
F32 = mybir.dt.float32
F32R = mybir.dt.float32r
I32 = mybir.dt.int32
AF = mybir.ActivationFunctionType
ALU = mybir.AluOpType


def _r(ap):
    """View an fp32 AP as float32r for full-rate PE matmuls."""
    return ap.bitcast(F32R)


def build_kernel():
    nc = bacc.Bacc("TRN2", target_bir_lowering=False, debug=False)

    enc_d = nc.dram_tensor("encoder_outputs", [BLOC, S, E], F32, kind="ExternalInput")
    h_d = nc.dram_tensor("decoder_hidden", [BLOC, D], F32, kind="ExternalInput")
    mask_d = nc.dram_tensor("mask", [BLOC, S], I32, kind="ExternalInput")
    wh_d = nc.dram_tensor("W_h", [D, E], F32, kind="ExternalInput")
    ws_d = nc.dram_tensor("W_s", [D, D], F32, kind="ExternalInput")
    v_d = nc.dram_tensor("v", [D], F32, kind="ExternalInput")
    ctx_d = nc.dram_tensor("context", [BLOC, E], F32, kind="ExternalOutput")
    attn_d = nc.dram_tensor("attn_weights", [BLOC, S], F32, kind="ExternalOutput")

    with tile.TileContext(nc) as tc:
        with (
            tc.tile_pool(name="singles", bufs=1) as sb,
            tc.tile_pool(name="psum", bufs=1, space="PSUM") as psum,
            tc.tile_pool(name="dramp", bufs=1, space="DRAM") as dram,
        ):
            ident = sb.tile([P, P], F32, name="ident")
            make_identity(nc, ident)

            # HAM warmup: a dense burst of real (non-transpose) matmuls so
            # the PE clock ungates early; fills the initial weight-DMA wait.
            warm_ps = psum.tile([P, SCH], F32, tag="pt", bufs=2, name="warm")
            for _ in range(16):
                nc.tensor.matmul(warm_ps[:, 0:P], lhsT=ident, rhs=ident,
                                 start=True, stop=True)

            # v striped: v_sb[p, o] = v[o*P + p]
            v_sb = sb.tile([P, DT], F32R, name="v_sb")
            nc.sync.dma_start(
                out=v_sb,
                in_=v_d[:].rearrange("(o p) -> p o", p=P).bitcast(F32R))

            # whT[p, e_t, d] = W_h[d, e_t*P + p]  -- persistent, 32KB/part
            whT = sb.tile([P, ET, D], F32R, name="whT")
            # hT[p, k, b] = h[b, k*P + p]
            hT = sb.tile([P, DT, BLOC], F32, name="hT")
            for o in range(DT):
                nc.sync.dma_start(
                    out=hT[:, o, :],
                    in_=h_d[:, o * P:(o + 1) * P].rearrange("b p -> p b"))
            # decT[p, o, b] = dec[b, o*P + p]
            decT = sb.tile([P, DT, BLOC], F32, name="decT")

            # ---------- interleaved setup + main loop ----------
            # One pool for everything; setup tiles share the streaming tags so
            # each phase's DMA loads during the previous phase's PE work.
            with tc.tile_pool(name="work", bufs=1) as work:
                wh_nat = wh_d[:].rearrange("(o p) e -> p o e", p=P)
                ws_nat = ws_d[:].rearrange("(o p) d -> p o d", p=P)

                def setup_wh():
                    for o2 in range(DT // 2):
                        wnat2 = work.tile([P, 2, E], F32, tag="nat", bufs=2,
                                          name="whload")
                        nc.scalar.dma_start(
                            out=wnat2, in_=wh_nat[:, 2 * o2:2 * o2 + 2, :])
                        for j in range(2):
                            o = 2 * o2 + j
                            for g in range(ET // 4):
                                pt = psum.tile([P, 4 * P], F32, tag="pt",
                                               bufs=2, name="ptw")
                                # keep-warm: transposes don't feed PE_HAM
                                nc.tensor.matmul(pt[:, 0:64], lhsT=ident,
                                                 rhs=ident[:, 0:64],
                                                 start=True, stop=True)
                                for k in range(4):
                                    e_t = 4 * g + k
                                    nc.tensor.transpose(
                                        pt[:, k * P:(k + 1) * P],
                                        wnat2[:, j, e_t * P:(e_t + 1) * P],
                                        ident)
                                dst = whT[:, 4 * g:4 * g + 4,
                                          o * P:(o + 1) * P]
                                ptv = pt.rearrange("p (k c) -> p k c", k=4)
                                if (o + g) % 2 == 0:
                                    nc.vector.tensor_copy(out=dst, in_=ptv)
                                else:
                                    nc.scalar.copy(out=dst, in_=ptv)

                def setup_ws_dec():
                    # W_s^T in two d-halves (both live), then dec per-o with
                    # a single accumulation region per PSUM tile.
                    halves = []
                    for h in range(2):
                        wsth = work.tile([P, 4, D], F32, tag="wsth", bufs=2,
                                         name=f"wsth{h}")
                        halves.append(wsth)
                        for o2 in range(4):
                            wnat2 = work.tile([P, 2, SCH], F32, tag="nat",
                                              bufs=2, name="wsload")
                            nc.scalar.dma_start(
                                out=wnat2,
                                in_=ws_nat[:, 2 * o2:2 * o2 + 2,
                                           h * SCH:(h + 1) * SCH])
                            for j in range(2):
                                eo = 2 * o2 + j
                                pt = psum.tile([P, 4 * P], F32, tag="pt",
                                               bufs=2, name="ptw")
                                nc.tensor.matmul(pt[:, 0:64], lhsT=ident,
                                                 rhs=ident[:, 0:64],
                                                 start=True, stop=True)
                                for kl in range(4):
                                    nc.tensor.transpose(
                                        pt[:, kl * P:(kl + 1) * P],
                                        wnat2[:, j, kl * P:(kl + 1) * P],
                                        ident)
                                dst = wsth[:, 0:4, eo * P:(eo + 1) * P]
                                ptv = pt.rearrange("p (k c) -> p k c", k=4)
                                if (eo + h) % 2 == 0:
                                    nc.vector.tensor_copy(out=dst, in_=ptv)
                                else:
                                    nc.scalar.copy(out=dst, in_=ptv)
                    for o in range(DT):
                        pdec = psum.tile([P, BLOC], F32, tag="pm", bufs=2,
                                         name="pdec")
                        for h in range(2):
                            for kl in range(4):
                                nc.tensor.matmul(
                                    pdec,
                                    lhsT=halves[h][:, kl, o * P:(o + 1) * P],
                                    rhs=hT[:, 4 * h + kl, :],
                                    start=(h == 0 and kl == 0),
                                    stop=(h == 1 and kl == 3),
                                )
                        nc.vector.tensor_copy(out=decT[:, o, :], in_=pdec)

                energy_t, rz_t, wcol_t = {}, {}, {}

                def chunk_trans(b, c):
                    nat = work.tile([P, SCH // P, E], F32, tag="nat",
                                    bufs=2, name=f"nat{b}_{c}")
                    s0 = c * SCH
                    nc.scalar.dma_start(
                        out=nat,
                        in_=enc_d[b, s0:s0 + SCH, :].rearrange(
                            "(st p) e -> p st e", p=P))
                    encT = work.tile([P, ET, SCH], F32R, tag="encT", bufs=2,
                                     name=f"encT{b}_{c}")
                    for e_t in range(ET):
                        pt = psum.tile([P, SCH], F32, tag="pt", bufs=2,
                                       name="ptc")
                        if e_t in (0, 4):
                            nc.tensor.matmul(pt[:, 0:64], lhsT=ident,
                                             rhs=ident[:, 0:64],
                                             start=True, stop=True)
                        for st in range(SCH // P):
                            nc.tensor.transpose(
                                pt[:, st * P:(st + 1) * P],
                                nat[:, st, e_t * P:(e_t + 1) * P],
                                ident,
                            )
                        if e_t % 2 == 0:
                            nc.vector.tensor_copy(out=encT[:, e_t, :], in_=pt)
                        else:
                            nc.scalar.copy(out=encT[:, e_t, :], in_=pt)
                    return encT

                def chunk_dloop(b, c, encT, energy):
                    pe_ = psum.tile([1, SCH], F32, tag="pe", bufs=2, name="pex")
                    for o in range(DT):
                        pm = psum.tile([P, SCH], F32, tag="pm", bufs=2,
                                       name="pmx")
                        for e_t in range(ET):
                            nc.tensor.matmul(
                                pm,
                                lhsT=whT[:, e_t, o * P:(o + 1) * P],
                                rhs=encT[:, e_t, :],
                                start=(e_t == 0),
                                stop=(e_t == ET - 1),
                            )
                        th = work.tile([P, SCH], F32R, tag="tanh", bufs=3,
                                       name="th")
                        nc.scalar.activation(
                            out=th, in_=pm, func=AF.Tanh,
                            bias=decT[:, o, b:b + 1], scale=1.0,
                        )
                        nc.tensor.matmul(
                            pe_,
                            lhsT=v_sb[:, o:o + 1],
                            rhs=th,
                            start=(o == 0),
                            stop=(o == DT - 1),
                        )
                    nc.vector.tensor_copy(
                        out=energy[:, c * SCH:(c + 1) * SCH], in_=pe_)

                def chunks(b):
                    energy = work.tile([1, S], F32, tag="energy", bufs=2,
                                       name=f"energy{b}")
                    energy_t[b] = energy
                    for c in range(NCHUNK):
                        chunk_dloop(b, c, chunk_trans(b, c), energy)

                def softmax(b):
                    energy = energy_t[b]
                    # mask row -> fp32 (loaded lazily to cap SBUF rows)
                    mi = work.tile([1, S], I32, tag="maski", bufs=1, name="mi")
                    nc.sync.dma_start(out=mi, in_=mask_d[b:b + 1, :])
                    mf = work.tile([1, S], F32, tag="maskf", bufs=1, name="mf")
                    nc.vector.tensor_copy(out=mf, in_=mi)

                    negmax = work.tile([1, 1], F32, tag="negmax", bufs=2,
                                       name=f"negmax{b}")
                    nc.vector.tensor_reduce(out=negmax, in_=energy,
                                            axis=mybir.AxisListType.X,
                                            op=ALU.max, negate=True)
                    wm = work.tile([1, S], F32, tag="wm", bufs=2, name=f"wm{b}")
                    nc.scalar.activation(out=wm, in_=energy, func=AF.Exp,
                                         bias=negmax, scale=1.0)
                    # masked unnormalized weights -> reuse the energy tile
                    zsum = work.tile([1, 1], F32, tag="zsum", bufs=2,
                                     name=f"z{b}")
                    nc.vector.tensor_mul(energy, wm, mf)
                    nc.vector.reduce_sum(zsum, energy,
                                         axis=mybir.AxisListType.X)
                    rz = work.tile([1, 1], F32, tag="rz", bufs=2, name=f"rz{b}")
                    nc.vector.reciprocal(rz, zsum)
                    rz_t[b] = rz
                    # attn = masked * rz, into the wm tile
                    nc.vector.tensor_scalar_mul(wm, energy, rz)
                    nc.sync.dma_start(out=attn_d[b:b + 1, :], in_=wm)
                    # unnormalized weights to column form via a DRAM bounce:
                    # wcol[p, t] = w_masked[t*P + p]
                    wrow = dram.tile([1, S], F32, tag="wrow", bufs=2,
                                     name=f"wrow{b}")
                    nc.sync.dma_start(out=wrow, in_=energy)
                    wcol = work.tile([P, ST], F32R, tag="wcol", bufs=2,
                                     name=f"wcol{b}")
                    nc.sync.dma_start(
                        out=wcol,
                        in_=wrow[:].rearrange("o (t p) -> (o p) t", p=P)
                        .bitcast(F32R))
                    wcol_t[b] = wcol

                def emit_reload(b, t2, tag, bufs):
                    # loads s-tiles 2*t2 and 2*t2+1 in one 1 MiB DMA
                    rl = work.tile([P, 2, E], F32R, tag=tag, bufs=bufs,
                                   name=f"{tag}_{b}_{t2}")
                    nc.scalar.dma_start(
                        out=rl,
                        in_=enc_d[b, 2 * t2 * P:(2 * t2 + 2) * P, :].rearrange(
                            "(q p) e -> p q e", p=P).bitcast(F32R))
                    return rl

                def context(b, rls=None):
                    wcol, rz = wcol_t[b], rz_t[b]
                    pc0 = psum.tile([1, SCH], F32, tag="pc", bufs=2, name="pc0")
                    pc1 = psum.tile([1, SCH], F32, tag="pc", bufs=2, name="pc1")
                    for t2 in range(ST // 2):
                        if rls is not None and t2 < len(rls):
                            rl = rls[t2]
                        else:
                            rl = emit_reload(b, t2, "rl", 2)
                        for q in range(2):
                            t = 2 * t2 + q
                            wc = wcol[:, t:t + 1]
                            nc.tensor.matmul(
                                pc0, lhsT=wc, rhs=rl[:, q, 0:SCH],
                                start=(t == 0), stop=(t == ST - 1))
                            nc.tensor.matmul(
                                pc1, lhsT=wc, rhs=rl[:, q, SCH:E],
                                start=(t == 0), stop=(t == ST - 1))
                    ctx_sb = work.tile([1, E], F32, tag="ctx", bufs=2,
                                       name=f"ctx{b}")
                    nc.scalar.activation(out=ctx_sb[:, 0:SCH], in_=pc0,
                                         func=AF.Copy, bias=0.0, scale=rz)
                    nc.scalar.activation(out=ctx_sb[:, SCH:E], in_=pc1,
                                         func=AF.Copy, bias=0.0, scale=rz)
                    nc.sync.dma_start(out=ctx_d[b:b + 1, :], in_=ctx_sb)

                # emission order: W_h setup -> chunk-0 transposes -> W_s/dec
                # setup -> chunk-0 compute, so every phase's DMA streams in
                # under the previous phase's PE work; then batch b's context
                # is deferred past batch b+1's heavy PE phase.
                setup_wh()
                energy0 = work.tile([1, S], F32, tag="energy", bufs=2,
                                    name="energy0")
                energy_t[0] = energy0
                encT00 = chunk_trans(0, 0)
                setup_ws_dec()
                chunk_dloop(0, 0, encT00, energy0)
                for c in range(1, NCHUNK):
                    chunk_dloop(0, c, chunk_trans(0, c), energy0)
                softmax(0)
                for b in range(1, BLOC):
                    chunks(b)
                    softmax(b)
                    context(b - 1)
                context(BLOC - 1)

    nc.compile()
    return nc


_NC = None


def _get_nc():
    global _NC
    if _NC is None:
        _NC = build_kernel()
    return _NC


def _make_in_maps(inputs):
    in_maps = []
    for i in range(NCORES):
        lo, hi = i * BLOC, (i + 1) * BLOC
        in_maps.append({
            "encoder_outputs": np.ascontiguousarray(
                inputs["encoder_outputs"][lo:hi], dtype=np.float32),
            "decoder_hidden": np.ascontiguousarray(
                inputs["decoder_hidden"][lo:hi], dtype=np.float32),
            "mask": np.ascontiguousarray(inputs["mask"][lo:hi], dtype=np.int32),
            "W_h": np.asarray(inputs["W_h"], dtype=np.float32),
            "W_s": np.asarray(inputs["W_s"], dtype=np.float32),
            "v": np.asarray(inputs["v"], dtype=np.float32),
        })
    return in_maps


def kernel(decoder_hidden, encoder_outputs, mask, W_h, W_s, v):
    nc = _get_nc()
    in_maps = _make_in_maps(dict(
        decoder_hidden=decoder_hidden, encoder_outputs=encoder_outputs,
        mask=mask, W_h=W_h, W_s=W_s, v=v))
    res = run_bass_kernel_spmd(nc, in_maps, core_ids=list(range(NCORES)))
    context = np.concatenate(
        [res.results[i]["context"] for i in range(NCORES)], axis=0)
    attn = np.concatenate(
        [res.results[i]["attn_weights"] for i in range(NCORES)], axis=0)
    return (context.astype(np.float32), attn.astype(np.float32))


# revision 19
# speedup vs baseline: 1.0747x; 1.0747x over previous
"""Bahdanau attention TRN2 Bass kernel.

Shapes (hardcoded): B=32, S=2048, ENC=DEC=1024, fp32.
Sharding: data-parallel over batch B across 8 NeuronCores (4 batches/core);
W_h / W_s / v replicated.

Per-core algorithm (b in 0..3):
  enc_T[d, s]  = sum_e W_h[d, e] * encoder[b, s, e]        (PE, fp32r)
                 encoder tiles transposed on-chip via PE transpose-mode
  t[d, s]      = tanh(enc_T[d, s] + dec[b, d])             (ACT, fused bias)
  energy[s]    = sum_d v[d] * t[d, s]                      (PE, M=1 matmul)
  w[s]         = exp(energy - max) * mask                  (ACT + DVE fused reduce)
  attn[s]      = w[s] / sum(w)                             (DVE)
  context[e]   = (sum_s w[s] * encoder[b, s, e]) / sum(w)  (PE, M=1 matmul)
"""

import numpy as np

try:
    import concourse.bacc as bacc
except ImportError:  # fallback when the axon site path isn't preloaded
    import sys
    sys.path.insert(0, "/opt/trn_rl_repo")
    import concourse.bacc as bacc
import concourse.bass as bass
import concourse.mybir as mybir
import concourse.tile as tile
from concourse.bass_utils import run_bass_kernel_spmd
from concourse.masks import make_identity

B, S, E, D = 32, 2048, 1024, 1024
NCORES = 8
BLOC = B // NCORES  # 4 batches per core
P = 128
SCH = 512           # s-chunk processed per inner iteration
NCHUNK = S // SCH   # 4
ET = E // P         # 8 e-tiles (contraction tiles for the big matmul)
DT = D // P         # 8 d-tiles
ST = S // P         # 16 s-tiles
F32 = mybir.dt.float32
F32R = mybir.dt.float32r
I32 = mybir.dt.int32
AF = mybir.ActivationFunctionType
ALU = mybir.AluOpType


def _r(ap):
    """View an fp32 AP as float32r for full-rate PE matmuls."""
    return ap.bitcast(F32R)


def build_kernel():
    nc = bacc.Bacc("TRN2", target_bir_lowering=False, debug=False)

    enc_d = nc.dram_tensor("encoder_outputs", [BLOC, S, E], F32, kind="ExternalInput")
    h_d = nc.dram_tensor("decoder_hidden", [BLOC, D], F32, kind="ExternalInput")
    mask_d = nc.dram_tensor("mask", [BLOC, S], I32, kind="ExternalInput")
    wh_d = nc.dram_tensor("W_h", [D, E], F32, kind="ExternalInput")
    ws_d = nc.dram_tensor("W_s", [D, D], F32, kind="ExternalInput")
    v_d = nc.dram_tensor("v", [D], F32, kind="ExternalInput")
    ctx_d = nc.dram_tensor("context", [BLOC, E], F32, kind="ExternalOutput")
    attn_d = nc.dram_tensor("attn_weights", [BLOC, S], F32, kind="ExternalOutput")

    with tile.TileContext(nc) as tc:
        with (
            tc.tile_pool(name="singles", bufs=1) as sb,
            tc.tile_pool(name="psum", bufs=1, space="PSUM") as psum,
            tc.tile_pool(name="dramp", bufs=1, space="DRAM") as dram,
        ):
            ident = sb.tile([P, P], F32, name="ident")
            make_identity(nc, ident)

            identr = sb.tile([P, P], F32R, name="identr")
            nc.vector.tensor_copy(out=identr, in_=ident)

            # HAM warmup: a dense burst of real (non-transpose) matmuls so
            # the PE clock ungates early; fills the initial weight-DMA wait.
            warm_ps = psum.tile([P, SCH], F32R, tag="pt", bufs=2, name="warm")
            for _ in range(16):
                nc.tensor.matmul(warm_ps[:, 0:P].bitcast(F32), lhsT=ident, rhs=ident,
                                 start=True, stop=True)

            # v striped: v_sb[p, o] = v[o*P + p]
            v_sb = sb.tile([P, DT], F32R, name="v_sb")
            nc.sync.dma_start(
                out=v_sb,
                in_=v_d[:].rearrange("(o p) -> p o", p=P).bitcast(F32R))

            # whT[p, e_t, d] = W_h[d, e_t*P + p]  -- persistent, 32KB/part
            whT = sb.tile([P, ET, D], F32R, name="whT")
            # hT[p, k, b] = h[b, k*P + p]
            hT = sb.tile([P, DT, BLOC], F32, name="hT")
            for o in range(DT):
                nc.sync.dma_start(
                    out=hT[:, o, :],
                    in_=h_d[:, o * P:(o + 1) * P].rearrange("b p -> p b"))
            # decT[p, o, b] = dec[b, o*P + p]
            decT = sb.tile([P, DT, BLOC], F32, name="decT")

            # ---------- setup (transient pool, released afterward) ----------
            with tc.tile_pool(name="setup", bufs=1) as sp:
                wh_nat = wh_d[:].rearrange("(o p) e -> p o e", p=P)
                for o2 in range(DT // 2):
                    wnat2 = sp.tile([P, 2, E], F32R, tag="wnat", bufs=3,
                                    name="wnat")
                    nc.scalar.dma_start(
                        out=wnat2,
                        in_=wh_nat[:, 2 * o2:2 * o2 + 2, :].bitcast(F32R))
                    for o, wnat in ((2 * o2, wnat2[:, 0, :]),
                                    (2 * o2 + 1, wnat2[:, 1, :])):
                     for g in range(ET // 4):
                        pt = psum.tile([P, 4 * P], F32R, tag="pt", bufs=2,
                                       name="ptw")
                        # keep-warm: transposes alone don't feed PE_HAM
                        nc.tensor.matmul(pt[:, 0:64].bitcast(F32), lhsT=ident,
                                         rhs=ident[:, 0:64],
                                         start=True, stop=True)
                        for j in range(4):
                            e_t = 4 * g + j
                            nc.tensor.transpose(
                                pt[:, j * P:(j + 1) * P],
                                wnat[:, e_t * P:(e_t + 1) * P], identr)
                        dst = whT[:, 4 * g:4 * g + 4, o * P:(o + 1) * P]
                        ptv = pt.rearrange("p (j c) -> p j c", j=4)
                        if (o + g) % 2 == 0:
                            nc.vector.tensor_copy(out=dst, in_=ptv)
                        else:
                            nc.scalar.copy(out=dst, in_=ptv)

                # wsT[p, d_t, e] = W_s[e, d_t*P + p] -- setup only
                wsT = sp.tile([P, DT, D], F32, name="wsT")
                ws_nat = ws_d[:].rearrange("(o p) d -> p o d", p=P)
                for o2 in range(DT // 2):
                    wnat2 = sp.tile([P, 2, D], F32R, tag="wnat", bufs=3,
                                    name="wnat")
                    nc.scalar.dma_start(
                        out=wnat2,
                        in_=ws_nat[:, 2 * o2:2 * o2 + 2, :].bitcast(F32R))
                    for o, wnat in ((2 * o2, wnat2[:, 0, :]),
                                    (2 * o2 + 1, wnat2[:, 1, :])):
                     for g in range(DT // 4):
                        pt = psum.tile([P, 4 * P], F32R, tag="pt", bufs=2,
                                       name="ptw")
                        nc.tensor.matmul(pt[:, 0:64].bitcast(F32), lhsT=ident,
                                         rhs=ident[:, 0:64],
                                         start=True, stop=True)
                        for j in range(4):
                            d_t = 4 * g + j
                            nc.tensor.transpose(
                                pt[:, j * P:(j + 1) * P],
                                wnat[:, d_t * P:(d_t + 1) * P], identr)
                        dst = wsT[:, 4 * g:4 * g + 4, o * P:(o + 1) * P]
                        ptv = pt.rearrange("p (j c) -> p j c", j=4)
                        if (o + g) % 2 == 0:
                            nc.vector.tensor_copy(out=dst, in_=ptv)
                        else:
                            nc.scalar.copy(out=dst, in_=ptv)

                # dec = h @ W_s^T (striped into decT)
                for o in range(DT):
                    pdec = psum.tile([P, BLOC], F32, tag="pm", bufs=2, name="pdec")
                    for k in range(DT):
                        nc.tensor.matmul(
                            pdec,
                            lhsT=wsT[:, k, o * P:(o + 1) * P],
                            rhs=hT[:, k, :],
                            start=(k == 0),
                            stop=(k == DT - 1),
                        )
                    nc.vector.tensor_copy(out=decT[:, o, :], in_=pdec)

            # ---------- main loop ----------
            with tc.tile_pool(name="work", bufs=1) as work:
                energy_t, rz_t, wcol_t = {}, {}, {}

                def chunks(b):
                    energy = work.tile([1, S], F32, tag="energy", bufs=2,
                                       name=f"energy{b}")
                    energy_t[b] = energy
                    for c in range(NCHUNK):
                        nat = work.tile([P, SCH // P, E], F32R, tag="nat",
                                        bufs=2, name=f"nat{b}_{c}")
                        s0 = c * SCH
                        nc.scalar.dma_start(
                            out=nat,
                            in_=enc_d[b, s0:s0 + SCH, :].rearrange(
                                "(st p) e -> p st e", p=P).bitcast(F32R))
                        nats = [nat[:, st, :] for st in range(SCH // P)]
                        # transpose chunk into encT[p, e_t, s]
                        encT = work.tile([P, ET, SCH], F32R, tag="encT", bufs=2,
                                         name=f"encT{b}_{c}")
                        for e_t in range(ET):
                            pt = psum.tile([P, SCH], F32R, tag="pt", bufs=2,
                                           name="ptc")
                            if e_t in (0, 4):
                                nc.tensor.matmul(pt[:, 0:64].bitcast(F32), lhsT=ident,
                                                 rhs=ident[:, 0:64],
                                                 start=True, stop=True)
                            for st in range(SCH // P):
                                nc.tensor.transpose(
                                    pt[:, st * P:(st + 1) * P],
                                    nats[st][:, e_t * P:(e_t + 1) * P],
                                    identr,
                                )
                            if e_t % 2 == 0:
                                nc.vector.tensor_copy(out=encT[:, e_t, :], in_=pt)
                            else:
                                nc.scalar.copy(out=encT[:, e_t, :], in_=pt)
                        # big matmul + tanh + v-reduction
                        pe_ = psum.tile([1, SCH], F32, tag="pe", bufs=2, name="pex")
                        for o in range(DT):
                            pm = psum.tile([P, SCH], F32, tag="pm", bufs=2,
                                           name="pmx")
                            for e_t in range(ET):
                                nc.tensor.matmul(
                                    pm,
                                    lhsT=whT[:, e_t, o * P:(o + 1) * P],
                                    rhs=encT[:, e_t, :],
                                    start=(e_t == 0),
                                    stop=(e_t == ET - 1),
                                )
                            th = work.tile([P, SCH], F32R, tag="tanh", bufs=3,
                                           name="th")
                            nc.scalar.activation(
                                out=th, in_=pm, func=AF.Tanh,
                                bias=decT[:, o, b:b + 1], scale=1.0,
                            )
                            nc.tensor.matmul(
                                pe_,
                                lhsT=v_sb[:, o:o + 1],
                                rhs=th,
                                start=(o == 0),
                                stop=(o == DT - 1),
                            )
                        nc.vector.tensor_copy(out=energy[:, c * SCH:(c + 1) * SCH],
                                              in_=pe_)

                def softmax(b):
                    energy = energy_t[b]
                    # mask row -> fp32 (loaded lazily to cap SBUF rows)
                    mi = work.tile([1, S], I32, tag="maski", bufs=1, name="mi")
                    nc.sync.dma_start(out=mi, in_=mask_d[b:b + 1, :])
                    mf = work.tile([1, S], F32, tag="maskf", bufs=1, name="mf")
                    nc.vector.tensor_copy(out=mf, in_=mi)

                    negmax = work.tile([1, 1], F32, tag="negmax", bufs=2,
                                       name=f"negmax{b}")
                    nc.vector.tensor_reduce(out=negmax, in_=energy,
                                            axis=mybir.AxisListType.X,
                                            op=ALU.max, negate=True)
                    wm = work.tile([1, S], F32, tag="wm", bufs=2, name=f"wm{b}")
                    nc.scalar.activation(out=wm, in_=energy, func=AF.Exp,
                                         bias=negmax, scale=1.0)
                    # masked unnormalized weights -> reuse the energy tile
                    zsum = work.tile([1, 1], F32, tag="zsum", bufs=2,
                                     name=f"z{b}")
                    nc.vector.tensor_mul(energy, wm, mf)
                    nc.vector.reduce_sum(zsum, energy,
                                         axis=mybir.AxisListType.X)
                    rz = work.tile([1, 1], F32, tag="rz", bufs=2, name=f"rz{b}")
                    nc.vector.reciprocal(rz, zsum)
                    rz_t[b] = rz
                    # attn = masked * rz, into the wm tile
                    nc.vector.tensor_scalar_mul(wm, energy, rz)
                    nc.sync.dma_start(out=attn_d[b:b + 1, :], in_=wm)
                    # unnormalized weights to column form via a DRAM bounce:
                    # wcol[p, t] = w_masked[t*P + p]
                    wrow = dram.tile([1, S], F32, tag="wrow", bufs=2,
                                     name=f"wrow{b}")
                    nc.sync.dma_start(out=wrow, in_=energy)
                    wcol = work.tile([P, ST], F32R, tag="wcol", bufs=2,
                                     name=f"wcol{b}")
                    nc.sync.dma_start(
                        out=wcol,
                        in_=wrow[:].rearrange("o (t p) -> (o p) t", p=P)
                        .bitcast(F32R))
                    wcol_t[b] = wcol

                def emit_reload(b, t2, tag, bufs):
                    # loads s-tiles 2*t2 and 2*t2+1 in one 1 MiB DMA
                    rl = work.tile([P, 2, E], F32R, tag=tag, bufs=bufs,
                                   name=f"{tag}_{b}_{t2}")
                    nc.scalar.dma_start(
                        out=rl,
                        in_=enc_d[b, 2 * t2 * P:(2 * t2 + 2) * P, :].rearrange(
                            "(q p) e -> p q e", p=P).bitcast(F32R))
                    return rl

                def context(b, rls=None):
                    wcol, rz = wcol_t[b], rz_t[b]
                    pc0 = psum.tile([1, SCH], F32, tag="pc", bufs=2, name="pc0")
                    pc1 = psum.tile([1, SCH], F32, tag="pc", bufs=2, name="pc1")
                    for t2 in range(ST // 2):
                        if rls is not None:
                            rl = rls[t2] if t2 < len(rls) else emit_reload(
                                b, t2, "rl3", 2)
                        else:
                            rl = emit_reload(b, t2, "rl", 2)
                        for q in range(2):
                            t = 2 * t2 + q
                            wc = wcol[:, t:t + 1]
                            nc.tensor.matmul(
                                pc0, lhsT=wc, rhs=rl[:, q, 0:SCH],
                                start=(t == 0), stop=(t == ST - 1))
                            nc.tensor.matmul(
                                pc1, lhsT=wc, rhs=rl[:, q, SCH:E],
                                start=(t == 0), stop=(t == ST - 1))
                    ctx_sb = work.tile([1, E], F32, tag="ctx", bufs=2,
                                       name=f"ctx{b}")
                    nc.scalar.activation(out=ctx_sb[:, 0:SCH], in_=pc0,
                                         func=AF.Copy, bias=0.0, scale=rz)
                    nc.scalar.activation(out=ctx_sb[:, SCH:E], in_=pc1,
                                         func=AF.Copy, bias=0.0, scale=rz)
                    nc.sync.dma_start(out=ctx_d[b:b + 1, :], in_=ctx_sb)

                # emission order: defer batch b's context past batch b+1's
                # heavy PE phase so the PE never waits on the softmax chain.
                for b in range(BLOC):
                    chunks(b)
                    if b == BLOC - 1:
                        # prefetch the last batch's first context reloads so
                        # its tail overlaps the softmax chain (only as many as
                        # there are slots -- more would stall the ACT queue)
                        last_rls = [emit_reload(b, t2, "rl3", 2)
                                    for t2 in range(2)]
                    softmax(b)
                    if b > 0:
                        context(b - 1)
                context(BLOC - 1, last_rls)

    nc.compile()
    return nc


_NC = None


def _get_nc():
    global _NC
    if _NC is None:
        _NC = build_kernel()
    return _NC


def _make_in_maps(inputs):
    in_maps = []
    for i in range(NCORES):
        lo, hi = i * BLOC, (i + 1) * BLOC
        in_maps.append({
            "encoder_outputs": np.ascontiguousarray(
                inputs["encoder_outputs"][lo:hi], dtype=np.float32),
            "decoder_hidden": np.ascontiguousarray(
                inputs["decoder_hidden"][lo:hi], dtype=np.float32),
            "mask": np.ascontiguousarray(inputs["mask"][lo:hi], dtype=np.int32),
            "W_h": np.asarray(inputs["W_h"], dtype=np.float32),
            "W_s": np.asarray(inputs["W_s"], dtype=np.float32),
            "v": np.asarray(inputs["v"], dtype=np.float32),
        })
    return in_maps


def kernel(decoder_hidden, encoder_outputs, mask, W_h, W_s, v):
    nc = _get_nc()
    in_maps = _make_in_maps(dict(
        decoder_hidden=decoder_hidden, encoder_outputs=encoder_outputs,
        mask=mask, W_h=W_h, W_s=W_s, v=v))
    res = run_bass_kernel_spmd(nc, in_maps, core_ids=list(range(NCORES)))
    context = np.concatenate(
        [res.results[i]["context"] for i in range(NCORES)], axis=0)
    attn = np.concatenate(
        [res.results[i]["attn_weights"] for i in range(NCORES)], axis=0)
    return (context.astype(np.float32), attn.astype(np.float32))
